# revision 13
# baseline (speedup 1.0000x reference)
"""Haar DWT (single-level) Trainium2 Bass kernel.

Input:  x (8, 32, 512, 512) float32
Output: (LL, LH, HL, HH), each (8, 32, 256, 256) float32

Sharding: pure data parallel over the batch dim — core b processes x[b].

Per-core algorithm (x_c: (32, 512, 512)):
  Flatten rows to (16384, 512). Process in blocks of G images
  (G*512 rows). Partition p holds K = G*512/128 consecutive rows
  (contiguous DRAM chunk -> efficient DMA).
  Stage 1 (row butterfly, DVE tensor_tensor):
      S = even_row + odd_row ; T = odd_row - even_row
  Stage 2 (column butterfly, DVE tensor_tensor_reduce, scale=0.25 folded):
      LL = 0.25*(S_e + S_o) ; HL = 0.25*(S_o - S_e)
      LH = 0.25*(T_e + T_o) ; HH = 0.25*(T_o - T_e)
"""

import sys

import numpy as np

if "/opt/trn_rl_repo" not in sys.path:
    sys.path.insert(0, "/opt/trn_rl_repo")

N_CORES = 8
C, H, W = 32, 512, 512
G = 2          # images per block
BX = 3         # input-tile pool buffers
BST = 3        # S/T pool buffers
BOUT = 4       # output-tile pool buffers
SPLIT_RINGS = True  # loads on SP HWDGE ring, stores on ACT HWDGE ring
P = 128

_PROGRAM = None


def _split_multi_waits(nc, mybir):
    """The walrus build in this image accepts at most ONE sync-wait per
    instruction ("Too many sync wait commands" otherwise). Tile's tail
    drain (and occasionally scheduled ops) carry several. Hoist excess
    waits onto single-wait NOPs inserted just before, on the same
    engine, preserving per-engine program order and semantics."""
    uid = 0
    for fn in nc.m.functions:
        for blk in fn.blocks:
            new_insts = []
            for inst in blk.instructions:
                si = getattr(inst, "sync_info", None)
                waits = list(si.on_wait) if si is not None and si.on_wait else []
                if len(waits) > 1:
                    for w in waits[:-1]:
                        uid += 1
                        nop = mybir.InstNoOp(
                            name=f"{inst.name}-swait{uid}",
                            engine=inst.engine,
                            sync_info=mybir.SyncInfo(on_wait=[w], on_update=[]),
                            bass_nofuse=True,
                        )
                        new_insts.append(nop)
                    si.on_wait = waits[-1:]
                new_insts.append(inst)
            blk.instructions[:] = new_insts


def _build_program():
    from concourse import bass, mybir
    from concourse.tile import TileContext

    f32 = mybir.dt.float32
    add = mybir.AluOpType.add
    sub = mybir.AluOpType.subtract

    NB = C // G          # blocks per core
    RIN = G * H          # input rows per block
    ROUT = G * (H // 2)  # output rows per block
    K = RIN // P         # input rows per partition (contiguous)
    Q = K // 2           # row pairs per partition
    M = W // 2

    nc = bass.Bass()
    x = nc.declare_dram_parameter("x", [C, H, W], f32, isOutput=False)
    outs = {
        nm: nc.declare_dram_parameter(nm, [C, H // 2, W // 2], f32, isOutput=True)
        for nm in ("LL", "LH", "HL", "HH")
    }

    xf = x[:].rearrange("c h w -> (c h) w")
    of = {nm: t[:].rearrange("c h w -> (c h) w") for nm, t in outs.items()}

    with TileContext(nc) as tc:
        with tc.tile_pool(name="pool", bufs=BUFS) as pool:
            for b in range(NB):
                X = pool.tile([P, K * W], f32, tag="X")
                src = xf[b * RIN:(b + 1) * RIN].rearrange(
                    "(p k) w -> p (k w)", p=P, k=K
                )
                nc.sync.dma_start(out=X[:], in_=src)

                Xv = X[:].rearrange("p (q e w) -> p q e w", q=Q, e=2, w=W)
                S = pool.tile([P, Q * W], f32, tag="S")
                T = pool.tile([P, Q * W], f32, tag="T")
                Sv = S[:].rearrange("p (q w) -> p q w", q=Q, w=W)
                Tv = T[:].rearrange("p (q w) -> p q w", q=Q, w=W)
                nc.vector.tensor_tensor(Sv, Xv[:, :, 0, :], Xv[:, :, 1, :], add)
                nc.vector.tensor_tensor(Tv, Xv[:, :, 1, :], Xv[:, :, 0, :], sub)

                # Fold the 0.25 on the (otherwise idle) scalar engine,
                # in place, while DVE moves on to other work.
                nc.scalar.mul(S[:], S[:], 0.25)
                nc.scalar.mul(T[:], T[:], 0.25)

                S4 = S[:].rearrange("p (q m e) -> p q m e", q=Q, m=M, e=2)
                T4 = T[:].rearrange("p (q m e) -> p q m e", q=Q, m=M, e=2)
                stage2 = {
                    "LL": (S4, 0, 1, add),
                    "HL": (S4, 1, 0, sub),
                    "LH": (T4, 0, 1, add),
                    "HH": (T4, 1, 0, sub),
                }
                for nm, (v, i0, i1, op) in stage2.items():
                    ot = pool.tile([P, Q * M], f32, tag=nm)
                    nc.vector.tensor_tensor(
                        ot[:].rearrange("p (q m) -> p q m", q=Q, m=M),
                        v[:, :, :, i0],
                        v[:, :, :, i1],
                        op,
                    )
                    dst = of[nm][b * ROUT:(b + 1) * ROUT].rearrange(
                        "(p k) w -> p (k w)", p=P, k=Q
                    )
                    st_eng = nc.scalar if SPLIT_RINGS else nc.sync
                    st_eng.dma_start(out=dst, in_=ot[:])

    _split_multi_waits(nc, mybir)
    return nc


def _get_program():
    global _PROGRAM
    if _PROGRAM is None:
        _PROGRAM = _build_program()
    return _PROGRAM


def _run(x, **spmd_kwargs):
    from concourse.bass_utils import run_bass_kernel_spmd

    nc = _get_program()
    in_maps = [
        {"x": np.ascontiguousarray(np.asarray(x)[b])} for b in range(N_CORES)
    ]
    res = run_bass_kernel_spmd(nc, in_maps, list(range(N_CORES)), **spmd_kwargs)
    full = {
        nm: np.stack([res.results[b][nm] for b in range(N_CORES)])
        for nm in ("LL", "LH", "HL", "HH")
    }
    return (full["LL"], full["LH"], full["HL"], full["HH"]), res


def kernel(x):
    out, _ = _run(x)
    return out


# revision 16
# speedup vs baseline: 1.0702x; 1.0702x over previous
"""Haar DWT (single-level) Trainium2 Bass kernel.

Input:  x (8, 32, 512, 512) float32
Output: (LL, LH, HL, HH), each (8, 32, 256, 256) float32

Sharding: pure data parallel over the batch dim — core b processes x[b].

Per-core algorithm (x_c: (32, 512, 512)):
  Flatten rows to (16384, 512). Process in blocks of G images
  (G*512 rows). Partition p holds K = G*512/128 consecutive rows
  (contiguous DRAM chunk -> efficient DMA).
  Stage 1 (row butterfly, DVE tensor_tensor):
      S = even_row + odd_row ; T = odd_row - even_row
  Stage 2 (column butterfly, DVE tensor_tensor_reduce, scale=0.25 folded):
      LL = 0.25*(S_e + S_o) ; HL = 0.25*(S_o - S_e)
      LH = 0.25*(T_e + T_o) ; HH = 0.25*(T_o - T_e)
"""

import sys

import numpy as np

if "/opt/trn_rl_repo" not in sys.path:
    sys.path.insert(0, "/opt/trn_rl_repo")

N_CORES = 8
C, H, W = 32, 512, 512
G = 2          # images per block
BX = 3         # input-tile pool buffers
BST = 3        # S/T pool buffers
BOUT = 4       # output-tile pool buffers
SPLIT_RINGS = True  # loads on SP HWDGE ring, stores on ACT HWDGE ring
P = 128

_PROGRAM = None


def _split_multi_waits(nc, mybir):
    """The walrus build in this image accepts at most ONE sync-wait per
    instruction ("Too many sync wait commands" otherwise). Tile's tail
    drain (and occasionally scheduled ops) carry several. Hoist excess
    waits onto single-wait NOPs inserted just before, on the same
    engine, preserving per-engine program order and semantics."""
    uid = 0
    for fn in nc.m.functions:
        for blk in fn.blocks:
            new_insts = []
            for inst in blk.instructions:
                si = getattr(inst, "sync_info", None)
                waits = list(si.on_wait) if si is not None and si.on_wait else []
                if len(waits) > 1:
                    for w in waits[:-1]:
                        uid += 1
                        nop = mybir.InstNoOp(
                            name=f"{inst.name}-swait{uid}",
                            engine=inst.engine,
                            sync_info=mybir.SyncInfo(on_wait=[w], on_update=[]),
                            bass_nofuse=True,
                        )
                        new_insts.append(nop)
                    si.on_wait = waits[-1:]
                new_insts.append(inst)
            blk.instructions[:] = new_insts


def _build_program():
    from concourse import bass, mybir
    from concourse.tile import TileContext

    f32 = mybir.dt.float32
    add = mybir.AluOpType.add
    sub = mybir.AluOpType.subtract

    # Heterogeneous blocks: small first/last blocks shorten the pipeline
    # ramp-in (first compute waits on the first load) and drain tail
    # (last stores wait on the last compute).
    img_blocks = [1, 1] + [G] * ((C - 4) // G) + [1, 1]
    assert sum(img_blocks) == C
    M = W // 2

    nc = bass.Bass()
    x = nc.declare_dram_parameter("x", [C, H, W], f32, isOutput=False)
    outs = {
        nm: nc.declare_dram_parameter(nm, [C, H // 2, W // 2], f32, isOutput=True)
        for nm in ("LL", "LH", "HL", "HH")
    }

    xf = x[:].rearrange("c h w -> (c h) w")
    of = {nm: t[:].rearrange("c h w -> (c h) w") for nm, t in outs.items()}

    with TileContext(nc) as tc:
        with tc.tile_pool(name="pool", bufs=BUFS) as pool:
            rin0 = 0
            rout0 = 0
            for gb in img_blocks:
                RIN = gb * H
                ROUT = gb * (H // 2)
                K = RIN // P
                Q = K // 2

                X = pool.tile([P, K * W], f32, tag="X")
                src = xf[rin0:rin0 + RIN].rearrange(
                    "(p k) w -> p (k w)", p=P, k=K
                )
                nc.sync.dma_start(out=X[:], in_=src)

                Xv = X[:].rearrange("p (q e w) -> p q e w", q=Q, e=2, w=W)
                S = pool.tile([P, Q * W], f32, tag="S")
                T = pool.tile([P, Q * W], f32, tag="T")
                Sv = S[:].rearrange("p (q w) -> p q w", q=Q, w=W)
                Tv = T[:].rearrange("p (q w) -> p q w", q=Q, w=W)
                nc.vector.tensor_tensor(Sv, Xv[:, :, 0, :], Xv[:, :, 1, :], add)
                nc.vector.tensor_tensor(Tv, Xv[:, :, 1, :], Xv[:, :, 0, :], sub)

                # Fold the 0.25 on the (otherwise idle) scalar engine,
                # in place, while DVE moves on to other work.
                nc.scalar.mul(S[:], S[:], 0.25)
                nc.scalar.mul(T[:], T[:], 0.25)

                S4 = S[:].rearrange("p (q m e) -> p q m e", q=Q, m=M, e=2)
                T4 = T[:].rearrange("p (q m e) -> p q m e", q=Q, m=M, e=2)
                stage2 = {
                    "LL": (S4, 0, 1, add),
                    "HL": (S4, 1, 0, sub),
                    "LH": (T4, 0, 1, add),
                    "HH": (T4, 1, 0, sub),
                }
                for nm, (v, i0, i1, op) in stage2.items():
                    ot = pool.tile([P, Q * M], f32, tag=nm)
                    nc.vector.tensor_tensor(
                        ot[:].rearrange("p (q m) -> p q m", q=Q, m=M),
                        v[:, :, :, i0],
                        v[:, :, :, i1],
                        op,
                    )
                    dst = of[nm][rout0:rout0 + ROUT].rearrange(
                        "(p k) w -> p (k w)", p=P, k=Q
                    )
                    st_eng = nc.scalar if SPLIT_RINGS else nc.sync
                    st_eng.dma_start(out=dst, in_=ot[:])

                rin0 += RIN
                rout0 += ROUT

    _split_multi_waits(nc, mybir)
    return nc


def _get_program():
    global _PROGRAM
    if _PROGRAM is None:
        _PROGRAM = _build_program()
    return _PROGRAM


def _run(x, **spmd_kwargs):
    from concourse.bass_utils import run_bass_kernel_spmd

    nc = _get_program()
    in_maps = [
        {"x": np.ascontiguousarray(np.asarray(x)[b])} for b in range(N_CORES)
    ]
    res = run_bass_kernel_spmd(nc, in_maps, list(range(N_CORES)), **spmd_kwargs)
    full = {
        nm: np.stack([res.results[b][nm] for b in range(N_CORES)])
        for nm in ("LL", "LH", "HL", "HH")
    }
    return (full["LL"], full["LH"], full["HL"], full["HH"]), res


def kernel(x):
    out, _ = _run(x)
    return out
